# revision 1
# baseline (speedup 1.0000x reference)
"""PVT-style spatial-reduction attention on 8 Trainium2 NeuronCores.

Sharding: data-parallel over batch (B=8 -> one batch element per core).
Each core runs the full attention for its batch element; weights are
replicated. No collectives needed.

Per-core pipeline (all matmuls fp16, accumulation fp32 in PSUM):
  x^T (via DMA-transpose)
  q^T = Wq^T x^T                     (lhsT=Wq chunks, rhs=x^T)
  xr  = conv2x2s2(x) = patches @ sr_w (lhsT=strided x^T views, rhs=sr_w)
  ln  = LayerNorm(xr + sr_b) * g + b
  ln^T (PE transpose)
  k^T = Wk^T ln^T ;  v = ln @ Wv     (v augmented with a ones column)
  per (head, q-block):  S^T = k^T.T q^T  (kv on partitions)
     P = exp(S*scale - 3)            (no row-max needed: scores ~ N(0,1))
     o = P^T.T @ [v | 1]             (lhsT = P slices; col 64 = row sums)
     attn = o[:, :64] / o[:, 64]
  out = attn @ Wp + bp               (lhsT = attn^T via PE transpose)
"""

import os
import sys
import numpy as np

for _p in ("/opt/trn_rl_repo", "/root/.axon_site/_ro/trn_rl_repo"):
    if os.path.isdir(_p) and _p not in sys.path:
        sys.path.append(_p)

import concourse.bacc as bacc
import concourse.bass as bass
import concourse.mybir as mybir
import concourse.tile as tile
from concourse.bass_utils import run_bass_kernel_spmd
from concourse.masks import make_identity

F16 = mybir.dt.float16
F32 = mybir.dt.float32

N = 4096          # q tokens (H*W = 64*64)
C = 320           # model dim
NH = 5            # heads
HD = 64           # head dim
NP = 1024         # kv tokens ((H/2)*(W/2))
QB = 512          # q block for scores
LN_EPS = 1e-3
SCALE = HD ** -0.5
EXP_BIAS = -3.0   # constant shift inside exp; cancels in softmax

# contraction chunks over C=320: three 128-row tiles; the last one holds
# c 192:320 and its top 64 rows overlap chunk 1 (use rows 64:128).
CCHUNKS = [(0, 0, 128), (128, 0, 128), (192, 64, 128)]  # (c_start, row0, rows)
# output chunks over C=320 (no overlap needed)
OCHUNKS = [(0, 128), (128, 128), (256, 64)]


def build_bass(dbg=False):
    nc = bacc.Bacc("TRN2", target_bir_lowering=False, debug=False, num_devices=8)

    x_d = nc.declare_dram_parameter("x", [N, C], F16, isOutput=False)
    wq_d = nc.declare_dram_parameter("wq", [C, C], F16, isOutput=False)
    wk_d = nc.declare_dram_parameter("wk", [C, C], F16, isOutput=False)
    wv_d = nc.declare_dram_parameter("wv", [C, C], F16, isOutput=False)
    srw_d = nc.declare_dram_parameter("srw", [4 * C, C], F16, isOutput=False)
    wp_d = nc.declare_dram_parameter("wp", [C, C], F16, isOutput=False)
    srb_d = nc.declare_dram_parameter("srb", [C], F32, isOutput=False)
    bk_d = nc.declare_dram_parameter("bk", [C], F32, isOutput=False)
    bv_d = nc.declare_dram_parameter("bv", [C], F32, isOutput=False)
    bp_d = nc.declare_dram_parameter("bp", [C], F32, isOutput=False)
    out_d = nc.declare_dram_parameter("out", [N, C], F32, isOutput=True)
    if dbg:
        dbg_d = {
            nm: nc.declare_dram_parameter(nm, shp, F16, isOutput=True)
            for nm, shp in [
                ("dbg_xt0", [128, N]), ("dbg_pat0", [128, NP]),
                ("dbg_ln0", [128, NP]), ("dbg_kt0", [128, NP]),
                ("dbg_v", [128, 8 * NH * 128]), ("dbg_qt0", [128, N]),
                ("dbg_se", [128, QB]), ("dbg_at", [128, 1024]),
            ]
        }

    with tile.TileContext(nc) as tc:
        with (
            tc.tile_pool(name="consts", bufs=1) as consts,
            tc.tile_pool(name="wpool", bufs=1) as wpool,
            tc.tile_pool(name="big", bufs=1) as bigp,
            tc.tile_pool(name="sexp", bufs=16) as sexp_p,
            tc.tile_pool(name="attn", bufs=2) as attn_p,
            tc.tile_pool(name="small", bufs=4) as small_p,
            tc.tile_pool(name="outp", bufs=4) as out_p,
            tc.tile_pool(name="ps_s", bufs=2, space="PSUM") as ps_s,
            tc.tile_pool(name="ps_a", bufs=2, space="PSUM") as ps_a,
            tc.tile_pool(name="ps_m", bufs=2, space="PSUM") as ps_m,
        ):
            # ---------------- x^T first (longest dependency chain) ----------
            xT = []
            for i, (c0, _r0, _rows) in enumerate(CCHUNKS):
                t = bigp.tile([128, N], F16, name=f"xT{i}")
                nc.sync.dma_start_transpose(out=t, in_=x_d[:, c0:c0 + 128])
                xT.append(t)

            # ---------------- constants / weights ----------------
            def bcast(dram_vec, name):
                t = consts.tile([128, C], F32, name=name)
                src = bass.AP(tensor=dram_vec.ap().tensor, offset=0,
                              ap=[[0, 128], [1, C]])
                nc.sync.dma_start(out=t, in_=src)
                return t

            srb_bc = bcast(srb_d, "srb_bc")
            bv_bc = bcast(bv_d, "bv_bc")
            bp_bc = bcast(bp_d, "bp_bc")

            ident = consts.tile([128, 128], F16, name="ident")
            make_identity(nc, ident)
            eps_t = consts.tile([128, 1], F32, name="eps_t")
            nc.vector.memset(eps_t, LN_EPS)
            ebias_t = consts.tile([128, 1], F32, name="ebias_t")
            nc.vector.memset(ebias_t, EXP_BIAS)

            def load_w_chunks(dram, name):
                ts = []
                for i, (c0, _r0, rows) in enumerate(CCHUNKS):
                    t = wpool.tile([rows, C], F16, name=f"{name}{i}")
                    nc.sync.dma_start(out=t, in_=dram[c0:c0 + rows, :])
                    ts.append(t)
                return ts

            wq_sb = load_w_chunks(wq_d, "wq")
            wk_sb = load_w_chunks(wk_d, "wk")
            wv_sb = load_w_chunks(wv_d, "wv")
            # Wp + bk in non-overlapping row chunks
            wp_o = []
            bk_col = []
            for i, (o0, osz) in enumerate(OCHUNKS):
                t = wpool.tile([osz, C], F16, name=f"wp{i}")
                nc.sync.dma_start(out=t, in_=wp_d[o0:o0 + osz, :])
                wp_o.append(t)
                b = wpool.tile([osz, 1], F32, name=f"bk{i}")
                nc.sync.dma_start(out=b, in_=bk_d[o0:o0 + osz].unsqueeze(1))
                bk_col.append(b)
            # sr_w in 10 even 128-row chunks of the flattened (dh,dw,ci) axis
            srw_sb = []
            for j in range(10):
                t = wpool.tile([128, C], F16, name=f"srw{j}")
                nc.sync.dma_start(out=t, in_=srw_d[j * 128:(j + 1) * 128, :])
                srw_sb.append(t)


            # v augmented: [128, kv_chunk(8), head(5), 96] with a ones
            # column at 0 (softmax denominators land on psum partition 0),
            # zeros at 1:32, and v at 32:96 (32-aligned partition base for
            # the normalization reads).
            VW = 128  # ones col 0, zero pad 1:64, v at 64:128
            v_aug = bigp.tile([128, 8, NH, VW], F16, name="v_aug")
            nc.vector.memset(v_aug[:, :, :, 0:64], 0.0)
            nc.vector.memset(v_aug[:, :, :, 0:1], 1.0)

            # conv patches in 10 even 128-row tiles matching srw chunks.
            # flat row f = s*320 + c maps to (dh,dw) = divmod(s,2) and
            # x^T chunk rows: c<128 -> xT0[c]; c<256 -> xT1[c-128];
            # else xT2[c-192] (its rows 64:128).
            def xt_src(c):
                if c < 128:
                    return 0, c
                if c < 256:
                    return 1, c - 128
                return 2, c - 192

            pat = []
            for j in range(10):
                t = bigp.tile([128, NP], F16, name=f"pat{j}")
                pat.append(t)
                f = j * 128
                while f < (j + 1) * 128:
                    s, c = divmod(f, C)
                    dh, dw = s // 2, s % 2
                    ci, r = xt_src(c)
                    run = min((j + 1) * 128 - f,          # tile rows left
                              C - c,                      # same s
                              (128 if c < 128 else 256 if c < 256 else 320) - c)
                    x3 = xT[ci].rearrange("p (h w) -> p h w", w=64)
                    p3 = pat[j].rearrange("p (h w) -> p h w", w=32)
                    for p in range(4):
                        nc.vector.tensor_copy(
                            out=p3[f - j * 128:f - j * 128 + run,
                                   8 * p:8 * p + 8, :],
                            in_=x3[r:r + run, 16 * p + dh:16 * p + 16:2,
                                   dw:64:2])
                    f += run

            # ---------------- pipelined conv/LN/q^T phase ----------------
            lnT = [bigp.tile([128, NP], F16, name=f"lnT{i}") for i in range(3)]
            ln_tiles = [None] * 8

            def emit_conv(it):
                pc = ps_m.tile([128, C], F32, name="pc", tag="m")
                for j in range(10):
                    nc.tensor.matmul(pc, pat[j][:, it * 128:(it + 1) * 128],
                                     srw_sb[j], start=(j == 0), stop=(j == 9))
                nc.vector.tensor_add(pc, pc, srb_bc)
                stats = small_p.tile([128, 6], F32, name="stats", tag="st")
                nc.vector.bn_stats(stats, pc)
                mv = small_p.tile([128, 2], F32, name="mv", tag="st")
                nc.vector.bn_aggr(mv, stats)
                std = small_p.tile([128, 1], F32, name="std", tag="st")
                nc.scalar.activation(std, mv[:, 1:2],
                                     mybir.ActivationFunctionType.Sqrt, bias=eps_t)
                rstd = small_p.tile([128, 1], F32, name="rstd", tag="st")
                nc.vector.reciprocal(rstd, std)
                ln_h = small_p.tile([128, C], F16, name="ln_h", tag="lnf")
                nc.vector.tensor_scalar(ln_h, pc, mv[:, 0:1], rstd,
                                        op0=mybir.AluOpType.subtract,
                                        op1=mybir.AluOpType.mult)
                ln_tiles[it] = ln_h

            def emit_lnT(it):
                ln_h = ln_tiles[it]
                for ci, (c0, _r0, rows) in enumerate(CCHUNKS):
                    pt = ps_a.tile([128, 128], F16, name="pt", tag="a")
                    nc.tensor.transpose(pt, ln_h[:, c0:c0 + 128], ident)
                    nc.vector.tensor_copy(lnT[ci][:, it * 128:(it + 1) * 128], pt)

            def emit_v(it):
                pv = ps_m.tile([128, C], F32, name="pv", tag="m")
                for ci, (_c0, r0, rows) in enumerate(CCHUNKS):
                    nc.tensor.matmul(pv, lnT[ci][r0:128, it * 128:(it + 1) * 128],
                                     wv_sb[ci][r0:128, :],
                                     start=(ci == 0), stop=(ci == 2))
                nc.vector.tensor_add(
                    v_aug[:, it, :, 64:],
                    pv.rearrange("p (h d) -> p h d", h=NH),
                    bv_bc.rearrange("p (h d) -> p h d", h=NH))

            qT = [bigp.tile([osz, N], F16, name=f"qT{i}")
                  for i, (_o0, osz) in enumerate(OCHUNKS)]

            def emit_qproj(nb):
                for i, (o0, osz) in enumerate(OCHUNKS):
                    pq = ps_s.tile([osz, QB], F32, name="pq", tag="s")
                    for ci, (_c0, r0, rows) in enumerate(CCHUNKS):
                        nc.tensor.matmul(
                            pq,
                            wq_sb[ci][r0:128, o0:o0 + osz],
                            xT[ci][r0:128, nb * QB:(nb + 1) * QB],
                            start=(ci == 0), stop=(ci == 2))
                    nc.vector.tensor_copy(qT[i][:, nb * QB:(nb + 1) * QB], pq)

            for it in range(8):
                emit_conv(it)
                emit_qproj(it)
                if it > 0:
                    emit_lnT(it - 1)
                    emit_v(it - 1)
            emit_lnT(7)
            emit_v(7)

            # k^T projection
            kT = []
            for i, (o0, osz) in enumerate(OCHUNKS):
                t = bigp.tile([osz, NP], F16, name=f"kT{i}")
                kT.append(t)
                for nb in range(NP // QB):
                    pk = ps_s.tile([osz, QB], F32, name="pk", tag="s")
                    for ci, (_c0, r0, rows) in enumerate(CCHUNKS):
                        nc.tensor.matmul(
                            pk,
                            wk_sb[ci][r0:128, o0:o0 + osz],
                            lnT[ci][r0:128, nb * QB:(nb + 1) * QB],
                            start=(ci == 0), stop=(ci == 2))
                    nc.vector.tensor_scalar_add(
                        t[:, nb * QB:(nb + 1) * QB], pk, bk_col[i])

            if dbg:
                nc.sync.dma_start(out=dbg_d["dbg_xt0"][:, :], in_=xT[0])
                nc.sync.dma_start(out=dbg_d["dbg_pat0"][:, :], in_=pat[0])
                nc.sync.dma_start(out=dbg_d["dbg_ln0"][:, :], in_=lnT[0])
                nc.sync.dma_start(out=dbg_d["dbg_kt0"][:, :], in_=kT[0])
                nc.sync.dma_start(out=dbg_d["dbg_v"][:, :],
                                  in_=v_aug.rearrange("p a b c -> p (a b c)"))
                nc.sync.dma_start(out=dbg_d["dbg_qt0"][:, :], in_=qT[0])

            # ---------------- attention + output projection ----------------
            # Scores are computed transposed (kv on partitions); attn@v with
            # V stationary yields attn^T (c on partitions) directly for the
            # projection. Heads are software-pipelined: scores/exp of head h
            # interleave with attn@v of head h-1 so the PE never waits for
            # the full exp batch (keeps the HAM clock-gate warm).
            QBB = 1024
            n_qb = N // QBB
            attnT = {}   # qb -> 3 chunk tiles
            s_exp = {}   # (qb, h) -> 8 exp tiles

            def emit_scores_block(qb, h, kvc):
                ht, hr = h // 2, (h % 2) * 64
                ps = ps_s.tile([128, QBB], F32, name="ps", tag="s")
                for qh in range(QBB // QB):
                    nc.tensor.matmul(
                        ps[:, qh * QB:(qh + 1) * QB],
                        kT[ht][hr:hr + HD, kvc * 128:(kvc + 1) * 128],
                        qT[ht][hr:hr + HD,
                               qb * QBB + qh * QB:qb * QBB + (qh + 1) * QB],
                        start=True, stop=True)
                se = sexp_p.tile([128, QBB], F16, name="se", tag="sexp")
                nc.scalar.activation(se, ps, mybir.ActivationFunctionType.Exp,
                                     bias=ebias_t, scale=SCALE)
                if dbg and qb == 0 and h == 0 and kvc == 0:
                    nc.sync.dma_start(out=dbg_d["dbg_se"][:, :], in_=se[:, :QB])
                s_exp[(qb, h)].append(se)

            def emit_norm(qb, h, qh, pav):
                dst = attnT[qb][h // 2]
                dr = (h % 2) * 64
                rec = small_p.tile([1, QB], F32, name="rec", tag="rc")
                nc.vector.reciprocal_approx_fast(rec, pav[0:1, :])
                rb = small_p.tile([HD, QB], F32, name="rb", tag="rb")
                nc.gpsimd.partition_broadcast(rb, rec)
                nc.vector.tensor_mul(
                    dst[dr:dr + HD, qh * QB:(qh + 1) * QB], pav[64:, :], rb)

            def emit_scores_av(qb, h, pqbh):
                """Interleave scores/exp of (qb, h) with attn@v of the
                previous (qb', h') at kv-chunk granularity, so the PE always
                has independent work while ACT catches up on exps."""
                s_exp[(qb, h)] = []
                attnT.setdefault(qb, [
                    attn_p.tile([osz, QBB], F16, name=f"aT{qb}_{i}",
                                tag=f"attn{i}")
                    for i, (_o0, osz) in enumerate(OCHUNKS)])
                pavs = None
                if pqbh is not None:
                    pavs = [ps_a.tile([128, QB], F32, name="pav", tag="a")
                            for _ in range(QBB // QB)]
                for kvc in range(8):
                    emit_scores_block(qb, h, kvc)
                    if pqbh is not None:
                        pq_, ph_ = pqbh
                        for qh in range(QBB // QB):
                            nc.tensor.matmul(
                                pavs[qh],
                                v_aug[:, kvc, ph_, :],
                                s_exp[(pq_, ph_)][kvc][:, qh * QB:(qh + 1) * QB],
                                start=(kvc == 0), stop=(kvc == 7))
                if pqbh is not None:
                    pq_, ph_ = pqbh
                    for qh in range(QBB // QB):
                        emit_norm(pq_, ph_, qh, pavs[qh])

            def emit_av_tail(qb, h):
                pavs = [ps_a.tile([128, QB], F32, name="pav", tag="a")
                        for _ in range(QBB // QB)]
                for kvc in range(8):
                    for qh in range(QBB // QB):
                        nc.tensor.matmul(
                            pavs[qh],
                            v_aug[:, kvc, h, :],
                            s_exp[(qb, h)][kvc][:, qh * QB:(qh + 1) * QB],
                            start=(kvc == 0), stop=(kvc == 7))
                for qh in range(QBB // QB):
                    emit_norm(qb, h, qh, pavs[qh])

            def emit_proj(qb):
                if dbg and qb == 0:
                    nc.sync.dma_start(out=dbg_d["dbg_at"][:, :], in_=attnT[0][0])
                for qs in range(QBB // 128):
                    po = ps_m.tile([128, C], F32, name="po", tag="m")
                    for ci, (o0, osz) in enumerate(OCHUNKS):
                        nc.tensor.matmul(
                            po, attnT[qb][ci][:, qs * 128:(qs + 1) * 128],
                            wp_o[ci], start=(ci == 0), stop=(ci == 2))
                    o_sb = out_p.tile([128, C], F32, name="o_sb", tag="o")
                    nc.vector.tensor_add(o_sb, po, bp_bc)
                    nc.sync.dma_start(
                        out=out_d[(qb * 8 + qs) * 128:(qb * 8 + qs + 1) * 128, :],
                        in_=o_sb)

            # pipelined emission: scores(i) interleaved with av(i-1);
            # proj(qb) after its last head's av
            work = [(qb, h) for qb in range(n_qb) for h in range(NH)]
            for i, (qb, h) in enumerate(work):
                pqbh = work[i - 1] if i > 0 else None
                emit_scores_av(qb, h, pqbh)
                if pqbh is not None and pqbh[1] == NH - 1:
                    emit_proj(pqbh[0])
            emit_av_tail(n_qb - 1, NH - 1)
            emit_proj(n_qb - 1)

    nc.compile()
    return nc


_CACHE = {}


def _get_nc():
    if "nc" not in _CACHE:
        _CACHE["nc"] = build_bass()
    return _CACHE["nc"]


def make_in_maps(x, Wq, Wkv, sr_w, sr_b, ln_g, ln_b, Wp, bp):
    B = x.shape[0]
    f16 = np.float16
    f32 = np.float32
    ln_g = np.asarray(ln_g, f32)
    ln_b = np.asarray(ln_b, f32)
    wk_f = np.asarray(Wkv[:, :C], f32)
    wv_f = np.asarray(Wkv[:, C:], f32)
    wq = np.ascontiguousarray(Wq, dtype=f16)
    # fold LN gamma/beta into the K/V projections:
    #   (ln*g + b) @ W = ln @ (g[:,None]*W) + b @ W
    wk = np.ascontiguousarray(ln_g[:, None] * wk_f, dtype=f16)
    wv = np.ascontiguousarray(ln_g[:, None] * wv_f, dtype=f16)
    bk = np.ascontiguousarray(ln_b @ wk_f, dtype=f32)
    bv = np.ascontiguousarray(ln_b @ wv_f, dtype=f32)
    srw = np.ascontiguousarray(np.asarray(sr_w, dtype=f16).reshape(4 * C, C))
    wp = np.ascontiguousarray(Wp, dtype=f16)
    srb = np.ascontiguousarray(sr_b, dtype=f32)
    bpv = np.ascontiguousarray(bp, dtype=f32)
    return [
        {"x": np.ascontiguousarray(x[i], dtype=f16), "wq": wq, "wk": wk,
         "wv": wv, "srw": srw, "wp": wp, "srb": srb, "bk": bk,
         "bv": bv, "bp": bpv}
        for i in range(B)
    ]


def kernel(x, Wq, Wkv, sr_w, sr_b, ln_g, ln_b, Wp, bp, H=64, W=64):
    x = np.asarray(x, dtype=np.float32)
    B = x.shape[0]
    assert x.shape == (B, N, C), x.shape
    nc = _get_nc()
    in_maps = make_in_maps(x, Wq, Wkv, sr_w, sr_b, ln_g, ln_b, Wp, bp)
    res = run_bass_kernel_spmd(nc, in_maps, core_ids=list(range(8)))
    out = np.stack([res.results[i]["out"] for i in range(B)], axis=0)
    return out.astype(np.float32)

